# revision 12
# baseline (speedup 1.0000x reference)
"""Chamfer-distance kernel for 8 Trainium2 NeuronCores (Bass/Tile).

Problem: gts [8, 8192, 3] f32, preds [8, 8192, 3] f32 ->
         scalar chamfer distance (pytorch3d convention: squared L2,
         mean over points, mean over batch, sum of both directions).

Sharding: one batch element per NeuronCore (B == n_cores == 8).

Per-core algorithm:
  - d2[n, m] = |x_n|^2 + |y_m|^2 - 2 x.y is produced by the PE as a single
    K=24 bf16 matmul per [128 x 512] tile: x/y coordinates are split into
    3-term bf16 hi/mid/lo components (fp32-class accuracy); |x|^2 / |y|^2
    enter as extra contraction rows against a ones-row.  The packed
    operands are replicated at partition offsets 0/32/64/96 so consecutive
    matmuls use different PE row-groups.
  - ScalarE drains every PSUM group with a plain f32->bf16 copy into a
    [128, 8192] bf16 row buffer (~1.9us per [128, 2048] group; ~492us
    total, the pipeline's critical engine).
  - VectorE does the column-direction reduction as one full-row
    elementwise min per tile (tensor_tensor, 2x bf16 mode) and the
    row-direction reduction as one tensor_scalar min-reduce per tile
    whose accumulator (accum = reduce(out, op1)) emits the row min at
    the 4x bf16 mode -- replacing the log-folding chain of the previous
    version.  DVE busy ~421us, hidden under ScalarE.
  - Host: final 128-partition column min + relu clamp + means (epilogue
    on tiny outputs).
"""

import sys

sys.path.insert(0, "/opt/trn_rl_repo")

import numpy as np
import ml_dtypes

import concourse.bacc as bacc
import concourse.mybir as mybir
import concourse.tile as tile
from concourse.bass_utils import run_bass_kernel_spmd

BF16 = ml_dtypes.bfloat16
POS_BIG = 3.0e38

B = 8
N_PTS = 8192
M_PTS = 8192
K_ROWS = 24
ROT = 4
H_ROWS = 32 * (ROT - 1) + K_ROWS
GROUP = 2048
BETA = 300.0


def _split3(v):
    """float64 vector -> three bf16-representable float64 components."""
    a0 = v.astype(BF16).astype(np.float64)
    a1 = (v - a0).astype(BF16).astype(np.float64)
    a2 = (v - a0 - a1).astype(BF16).astype(np.float64)
    return a0, a1, a2


def _host_pack(x, y):
    """x [N,3] f32, y [M,3] f32 -> xp [H,N] bf16, yp [H,M] bf16 such that
    sum_k xp[k,n]*yp[k,m] = d2(x_n, y_m), replicated at 4 row offsets."""
    N, M = x.shape[0], y.shape[0]
    xd = x.astype(np.float64)
    yd = y.astype(np.float64)
    ax = [_split3(xd[:, d]) for d in range(3)]
    by = [_split3(yd[:, d]) for d in range(3)]
    x2 = _split3((xd * xd).sum(1))
    y2 = _split3((yd * yd).sum(1))

    xp = np.zeros((K_ROWS, N), dtype=BF16)
    yp = np.zeros((K_ROWS, M), dtype=BF16)
    r = 0
    for (i, j) in [(0, 0), (0, 1), (1, 0), (1, 1), (0, 2), (2, 0)]:
        for d in range(3):
            xp[r] = (-2.0 * ax[d][i]).astype(BF16)
            yp[r] = by[d][j].astype(BF16)
            r += 1
    for k in range(3):
        xp[r] = x2[k].astype(BF16)
        yp[r] = np.ones(M, dtype=BF16)
        r += 1
    for k in range(3):
        xp[r] = np.ones(N, dtype=BF16)
        yp[r] = y2[k].astype(BF16)
        r += 1
    assert r == K_ROWS

    xr = np.zeros((H_ROWS, N), dtype=BF16)
    yr = np.zeros((H_ROWS, M), dtype=BF16)
    for q in range(ROT):
        xr[32 * q:32 * q + K_ROWS] = xp
        yr[32 * q:32 * q + K_ROWS] = yp
    return xr, yr


# Drain-engine split: simming showed any DVE-drained psum groups stall the
# in-order DVE queue (colmin/rowmin ops) and lose ~10us net, so ScalarE
# drains everything and VectorE keeps only the min-reduction work.
DVE_EVERY = 1 << 30


def _build_nc(reps=1):
    f32 = mybir.dt.float32
    bf16 = mybir.dt.bfloat16
    MIN = mybir.AluOpType.min
    nc = bacc.Bacc()
    n_tiles = N_PTS // 128
    n_groups = M_PTS // GROUP
    cpg = GROUP // 512

    xp = nc.declare_dram_parameter("xp", [H_ROWS, N_PTS], bf16, isOutput=False)
    yp = nc.declare_dram_parameter("yp", [H_ROWS, M_PTS], bf16, isOutput=False)
    rowout = nc.declare_dram_parameter("rowout", [128, n_tiles], f32, isOutput=True)
    colout = nc.declare_dram_parameter("colout", [128, M_PTS], bf16, isOutput=True)

    with tile.TileContext(nc) as tc:
        with (
            tc.tile_pool(name="inputs", bufs=1) as inp,
            tc.tile_pool(name="acc", bufs=1) as acc,
            tc.tile_pool(name="rows", bufs=4) as rows,
            tc.tile_pool(name="ps", bufs=2, space="PSUM") as psp,
        ):
            xs = inp.tile([H_ROWS, N_PTS], bf16)
            ys = inp.tile([H_ROWS, M_PTS], bf16)
            nc.sync.dma_start(xs[:], xp[:])
            nc.sync.dma_start(ys[:], yp[:])

            colmin = acc.tile([128, M_PTS], bf16)
            rowstash = acc.tile([128, n_tiles], f32)

            def body():
                for t in range(n_tiles):
                    row = rows.tile([128, M_PTS], bf16, name="row")
                    for g in range(n_groups):
                        ps = psp.tile([128, GROUP], f32, name="ps")
                        for j in range(cpg):
                            c = g * cpg + j
                            lo = 32 * (c % ROT)
                            nc.tensor.matmul(
                                ps[:, j * 512:(j + 1) * 512],
                                xs[lo:lo + K_ROWS, t * 128:(t + 1) * 128],
                                ys[lo:lo + K_ROWS, c * 512:(c + 1) * 512],
                                start=True,
                                stop=True,
                                tile_position=(lo, 0),
                            )
                        rs = row[:, g * GROUP:(g + 1) * GROUP]
                        if (t * n_groups + g) % DVE_EVERY == DVE_EVERY - 1:
                            nc.vector.tensor_copy(rs, ps[:])
                        else:
                            nc.scalar.copy(rs, ps[:])
                    # column direction: one full-row min-accumulate per tile
                    if t == 0:
                        nc.vector.tensor_copy(colmin[:], row[:])
                    else:
                        nc.vector.tensor_tensor(colmin[:], row[:], colmin[:],
                                                op=MIN)
                    # row direction: one 4x-mode min-reduce per tile via the
                    # tensor_scalar accumulator; out is an in-place identity
                    nc.vector.tensor_scalar(
                        row[:], row[:], POS_BIG, None, op0=MIN, op1=MIN,
                        accum_out=rowstash[:, t:t + 1])

            if reps == 1:
                body()
            else:
                with tc.For_i(0, reps, 1):
                    body()

            nc.sync.dma_start(rowout[:], rowstash[:])
            nc.sync.dma_start(colout[:], colmin[:])
    nc.compile()
    return nc


_CACHED_NC = None


def _get_nc():
    global _CACHED_NC
    if _CACHED_NC is None:
        _CACHED_NC = _build_nc(reps=1)
    return _CACHED_NC


def kernel(gts, preds):
    gts = np.asarray(gts, dtype=np.float32)
    preds = np.asarray(preds, dtype=np.float32)
    assert gts.shape == (B, N_PTS, 3) and preds.shape == (B, M_PTS, 3), (
        gts.shape, preds.shape)

    nc = _get_nc()
    in_maps = []
    for b in range(B):
        xp, yp = _host_pack(gts[b], preds[b])
        in_maps.append({"xp": xp, "yp": yp})

    res = run_bass_kernel_spmd(nc, in_maps, list(range(B)))

    total = 0.0
    for b in range(B):
        rmin = res.results[b]["rowout"].astype(np.float64)   # [128, n_tiles]
        cmin = res.results[b]["colout"].astype(np.float64)   # [128, M]
        sx = np.maximum(rmin, 0.0).sum()
        sy = np.maximum(cmin.min(axis=0), 0.0).sum()
        total += sx / N_PTS + sy / M_PTS
    return np.float32(total / B)


# revision 14
# speedup vs baseline: 1.3660x; 1.3660x over previous
"""Chamfer-distance kernel for 8 Trainium2 NeuronCores (Bass/Tile).

Problem: gts [8, 8192, 3] f32, preds [8, 8192, 3] f32 ->
         scalar chamfer distance (pytorch3d convention: squared L2,
         mean over points, mean over batch, sum of both directions).

Sharding: one batch element per NeuronCore (B == n_cores == 8).

Per-core algorithm (min convention):
  - d2[n, m] = |x_n|^2 + |y_m|^2 - 2 x.y is produced by the PE as a single
    K=24 bf16 matmul per [128 x 512] tile: x/y coordinates are split into
    3-term bf16 hi/mid/lo components (fp32-class accuracy); |x|^2 / |y|^2
    enter as extra contraction rows against a ones-row.  The packed
    operands are replicated at partition offsets 0/32/64/96 so consecutive
    matmuls use different PE row-groups (hides LDWEIGHTS, enables
    concurrent sub-array execution).
  - ScalarE copies each PSUM group [128, 2048] to a bf16 row buffer.
  - VectorE keeps a running elementwise column-min over n-tiles
    (colmin [128, 8192] bf16) and computes row-mins by a halving
    bf16 tensor-tensor min fold over each row block.
  - Host: means + relu clamp + batch mean (epilogue on tiny outputs).
"""

import sys

sys.path.insert(0, "/opt/trn_rl_repo")

import numpy as np
import ml_dtypes

import concourse.bacc as bacc
import concourse.mybir as mybir
import concourse.tile as tile
from concourse.bass_utils import run_bass_kernel_spmd

BF16 = ml_dtypes.bfloat16
POS_BIG = 3.0e38

B = 8
N_PTS = 8192
M_PTS = 8192
K_ROWS = 24
ROT = 4
H_ROWS = 32 * (ROT - 1) + K_ROWS
GROUP = 2048


def _split3(v):
    """float64 vector -> three bf16-representable float64 components."""
    a0 = v.astype(BF16).astype(np.float64)
    a1 = (v - a0).astype(BF16).astype(np.float64)
    a2 = (v - a0 - a1).astype(BF16).astype(np.float64)
    return a0, a1, a2


def _host_pack(x, y):
    """x [N,3] f32, y [M,3] f32 -> xp [H,N] bf16, yp [H,M] bf16 such that
    sum_k xp[k,n]*yp[k,m] = d2(x_n, y_m), replicated at 4 row offsets."""
    N, M = x.shape[0], y.shape[0]
    xd = x.astype(np.float64)
    yd = y.astype(np.float64)
    ax = [_split3(xd[:, d]) for d in range(3)]
    by = [_split3(yd[:, d]) for d in range(3)]
    x2 = _split3((xd * xd).sum(1))
    y2 = _split3((yd * yd).sum(1))

    xp = np.zeros((K_ROWS, N), dtype=BF16)
    yp = np.zeros((K_ROWS, M), dtype=BF16)
    r = 0
    for (i, j) in [(0, 0), (0, 1), (1, 0), (1, 1), (0, 2), (2, 0)]:
        for d in range(3):
            xp[r] = (-2.0 * ax[d][i]).astype(BF16)
            yp[r] = by[d][j].astype(BF16)
            r += 1
    for k in range(3):
        xp[r] = x2[k].astype(BF16)
        yp[r] = np.ones(M, dtype=BF16)
        r += 1
    for k in range(3):
        xp[r] = np.ones(N, dtype=BF16)
        yp[r] = y2[k].astype(BF16)
        r += 1
    assert r == K_ROWS

    xr = np.zeros((H_ROWS, N), dtype=BF16)
    yr = np.zeros((H_ROWS, M), dtype=BF16)
    for q in range(ROT):
        xr[32 * q:32 * q + K_ROWS] = xp
        yr[32 * q:32 * q + K_ROWS] = yp
    return xr, yr


def _build_nc(reps=1):
    f32 = mybir.dt.float32
    bf16 = mybir.dt.bfloat16
    MIN = mybir.AluOpType.min
    nc = bacc.Bacc()
    n_tiles = N_PTS // 128
    n_groups = M_PTS // GROUP
    cpg = GROUP // 512

    xp = nc.declare_dram_parameter("xp", [H_ROWS, N_PTS], bf16, isOutput=False)
    yp = nc.declare_dram_parameter("yp", [H_ROWS, M_PTS], bf16, isOutput=False)
    rowout = nc.declare_dram_parameter("rowout", [128, n_tiles], f32, isOutput=True)
    colout = nc.declare_dram_parameter("colout", [128, M_PTS], bf16, isOutput=True)

    with tile.TileContext(nc) as tc:
        with (
            tc.tile_pool(name="inputs", bufs=1) as inp,
            tc.tile_pool(name="acc", bufs=1) as acc,
            tc.tile_pool(name="cp", bufs=3) as cpp,
            tc.tile_pool(name="ps", bufs=2, space="PSUM") as psp,
        ):
            xs = inp.tile([H_ROWS, N_PTS], bf16)
            ys = inp.tile([H_ROWS, M_PTS], bf16)
            nc.sync.dma_start(xs[:], xp[:])
            nc.sync.dma_start(ys[:], yp[:])

            colmin = acc.tile([128, M_PTS], bf16)
            rowstash = acc.tile([128, n_tiles], f32)

            def body():
                for t in range(n_tiles):
                    cbrow = cpp.tile([128, M_PTS], bf16, name="cbrow")
                    for g in range(n_groups):
                        ps = psp.tile([128, GROUP], f32, name="ps")
                        for j in range(cpg):
                            c = g * cpg + j
                            lo = 32 * (c % ROT)
                            nc.tensor.matmul(
                                ps[:, j * 512:(j + 1) * 512],
                                xs[lo:lo + K_ROWS, t * 128:(t + 1) * 128],
                                ys[lo:lo + K_ROWS, c * 512:(c + 1) * 512],
                                start=True,
                                stop=True,
                                tile_position=(lo, 0),
                            )
                        nc.scalar.copy(cbrow[:, g * GROUP:(g + 1) * GROUP],
                                       ps[:])
                    # one full-row column-min accumulate per tile (fewer
                    # DVE instruction overheads than per-group updates)
                    if t == 0:
                        nc.vector.tensor_copy(colmin[:], cbrow[:])
                    else:
                        nc.vector.tensor_tensor(colmin[:], cbrow[:],
                                                colmin[:], op=MIN)
                    w = M_PTS // 2
                    while w >= 512:
                        nc.vector.tensor_tensor(
                            cbrow[:, :w], cbrow[:, :w], cbrow[:, w:2 * w], op=MIN)
                        w //= 2
                    nc.vector.tensor_reduce(
                        rowstash[:, t:t + 1], cbrow[:, :512],
                        axis=mybir.AxisListType.X, op=MIN,
                    )

            if reps == 1:
                body()
            else:
                with tc.For_i(0, reps, 1):
                    body()

            nc.sync.dma_start(rowout[:], rowstash[:])
            nc.sync.dma_start(colout[:], colmin[:])
    nc.compile()
    return nc


_CACHED_NC = None


def _get_nc():
    global _CACHED_NC
    if _CACHED_NC is None:
        _CACHED_NC = _build_nc(reps=1)
    return _CACHED_NC


def kernel(gts, preds):
    gts = np.asarray(gts, dtype=np.float32)
    preds = np.asarray(preds, dtype=np.float32)
    assert gts.shape == (B, N_PTS, 3) and preds.shape == (B, M_PTS, 3), (
        gts.shape, preds.shape)

    nc = _get_nc()
    in_maps = []
    for b in range(B):
        xp, yp = _host_pack(gts[b], preds[b])
        in_maps.append({"xp": xp, "yp": yp})

    res = run_bass_kernel_spmd(nc, in_maps, list(range(B)))

    total = 0.0
    for b in range(B):
        rmin = res.results[b]["rowout"].astype(np.float64)   # [128, n_tiles]
        cmin = res.results[b]["colout"].astype(np.float64)   # [128, M]
        sx = np.maximum(rmin, 0.0).sum()
        sy = np.maximum(cmin.min(axis=0), 0.0).sum()
        total += sx / N_PTS + sy / M_PTS
    return np.float32(total / B)



# revision 15
# speedup vs baseline: 1.4401x; 1.0542x over previous
"""Chamfer-distance kernel for 8 Trainium2 NeuronCores (Bass/Tile).

Problem: gts [8, 8192, 3] f32, preds [8, 8192, 3] f32 ->
         scalar chamfer distance (pytorch3d convention: squared L2,
         mean over points, mean over batch, sum of both directions).

Sharding: one batch element per NeuronCore (B == n_cores == 8).

Per-core algorithm (min convention):
  - d2[n, m] = |x_n|^2 + |y_m|^2 - 2 x.y is produced by the PE as a single
    K=24 bf16 matmul per [128 x 512] tile: x/y coordinates are split into
    3-term bf16 hi/mid/lo components (fp32-class accuracy); |x|^2 / |y|^2
    enter as extra contraction rows against a ones-row.  The packed
    operands are replicated at partition offsets 0/32/64/96 so consecutive
    matmuls use different PE row-groups (hides LDWEIGHTS, enables
    concurrent sub-array execution).
  - ScalarE copies each PSUM group [128, 2048] to a bf16 row buffer.
  - VectorE keeps a running elementwise column-min over n-tiles
    (colmin [128, 8192] bf16) and computes row-mins by a halving
    bf16 tensor-tensor min fold over each row block.
  - Host: means + relu clamp + batch mean (epilogue on tiny outputs).
"""

import sys

sys.path.insert(0, "/opt/trn_rl_repo")

import numpy as np
import ml_dtypes

import concourse.bacc as bacc
import concourse.mybir as mybir
import concourse.tile as tile
from concourse.bass_utils import run_bass_kernel_spmd

BF16 = ml_dtypes.bfloat16
POS_BIG = 3.0e38

B = 8
N_PTS = 8192
M_PTS = 8192
K_ROWS = 24
ROT = 4
H_ROWS = 32 * (ROT - 1) + K_ROWS
GROUP = 2048


def _split3(v):
    """float64 vector -> three bf16-representable float64 components."""
    a0 = v.astype(BF16).astype(np.float64)
    a1 = (v - a0).astype(BF16).astype(np.float64)
    a2 = (v - a0 - a1).astype(BF16).astype(np.float64)
    return a0, a1, a2


def _host_pack(x, y):
    """x [N,3] f32, y [M,3] f32 -> xp [H,N] bf16, yp [H,M] bf16 such that
    sum_k xp[k,n]*yp[k,m] = d2(x_n, y_m), replicated at 4 row offsets."""
    N, M = x.shape[0], y.shape[0]
    xd = x.astype(np.float64)
    yd = y.astype(np.float64)
    ax = [_split3(xd[:, d]) for d in range(3)]
    by = [_split3(yd[:, d]) for d in range(3)]
    x2 = _split3((xd * xd).sum(1))
    y2 = _split3((yd * yd).sum(1))

    xp = np.zeros((K_ROWS, N), dtype=BF16)
    yp = np.zeros((K_ROWS, M), dtype=BF16)
    r = 0
    for (i, j) in [(0, 0), (0, 1), (1, 0), (1, 1), (0, 2), (2, 0)]:
        for d in range(3):
            xp[r] = (-2.0 * ax[d][i]).astype(BF16)
            yp[r] = by[d][j].astype(BF16)
            r += 1
    for k in range(3):
        xp[r] = x2[k].astype(BF16)
        yp[r] = np.ones(M, dtype=BF16)
        r += 1
    for k in range(3):
        xp[r] = np.ones(N, dtype=BF16)
        yp[r] = y2[k].astype(BF16)
        r += 1
    assert r == K_ROWS

    xr = np.zeros((H_ROWS, N), dtype=BF16)
    yr = np.zeros((H_ROWS, M), dtype=BF16)
    for q in range(ROT):
        xr[32 * q:32 * q + K_ROWS] = xp
        yr[32 * q:32 * q + K_ROWS] = yp
    return xr, yr


def _build_nc(reps=1):
    f32 = mybir.dt.float32
    bf16 = mybir.dt.bfloat16
    MIN = mybir.AluOpType.min
    nc = bacc.Bacc()
    n_tiles = N_PTS // 128
    n_groups = M_PTS // GROUP
    cpg = GROUP // 512

    xp = nc.declare_dram_parameter("xp", [H_ROWS, N_PTS], bf16, isOutput=False)
    yp = nc.declare_dram_parameter("yp", [H_ROWS, M_PTS], bf16, isOutput=False)
    rowout = nc.declare_dram_parameter("rowout", [128, n_tiles], f32, isOutput=True)
    colout = nc.declare_dram_parameter("colout", [128, M_PTS], bf16, isOutput=True)

    with tile.TileContext(nc) as tc:
        with (
            tc.tile_pool(name="inputs", bufs=1) as inp,
            tc.tile_pool(name="acc", bufs=1) as acc,
            tc.tile_pool(name="cp", bufs=3) as cpp,
            tc.tile_pool(name="ps", bufs=2, space="PSUM") as psp,
        ):
            xs = inp.tile([H_ROWS, N_PTS], bf16)
            ys = inp.tile([H_ROWS, M_PTS], bf16)
            nc.sync.dma_start(xs[:], xp[:])
            nc.sync.dma_start(ys[:], yp[:])

            colmin = acc.tile([128, M_PTS], bf16)
            rowstash = acc.tile([128, n_tiles], f32)

            def body():
                for t in range(n_tiles):
                    cbrow = cpp.tile([128, M_PTS], bf16, name="cbrow")
                    for g in range(n_groups):
                        ps = psp.tile([128, GROUP], f32, name="ps")
                        for j in range(cpg):
                            c = g * cpg + j
                            lo = 32 * (c % ROT)
                            nc.tensor.matmul(
                                ps[:, j * 512:(j + 1) * 512],
                                xs[lo:lo + K_ROWS, t * 128:(t + 1) * 128],
                                ys[lo:lo + K_ROWS, c * 512:(c + 1) * 512],
                                start=True,
                                stop=True,
                                tile_position=(lo, 0),
                            )
                        cs = cbrow[:, g * GROUP:(g + 1) * GROUP]
                        nc.scalar.copy(cs, ps[:])
                        cm = colmin[:, g * GROUP:(g + 1) * GROUP]
                        if t == 0:
                            nc.vector.tensor_copy(cm, cs)
                        else:
                            nc.vector.tensor_tensor(cm, cs, cm, op=MIN)
                    w = M_PTS // 2
                    while w >= 512:
                        nc.vector.tensor_tensor(
                            cbrow[:, :w], cbrow[:, :w], cbrow[:, w:2 * w], op=MIN)
                        w //= 2
                    nc.vector.tensor_reduce(
                        rowstash[:, t:t + 1], cbrow[:, :512],
                        axis=mybir.AxisListType.X, op=MIN,
                    )

            if reps == 1:
                body()
            else:
                with tc.For_i(0, reps, 1):
                    body()

            nc.sync.dma_start(rowout[:], rowstash[:])
            nc.sync.dma_start(colout[:], colmin[:])
    nc.compile()
    return nc


_CACHED_NC = None


def _get_nc():
    global _CACHED_NC
    if _CACHED_NC is None:
        _CACHED_NC = _build_nc(reps=1)
    return _CACHED_NC


def kernel(gts, preds):
    gts = np.asarray(gts, dtype=np.float32)
    preds = np.asarray(preds, dtype=np.float32)
    assert gts.shape == (B, N_PTS, 3) and preds.shape == (B, M_PTS, 3), (
        gts.shape, preds.shape)

    nc = _get_nc()
    in_maps = []
    for b in range(B):
        xp, yp = _host_pack(gts[b], preds[b])
        in_maps.append({"xp": xp, "yp": yp})

    res = run_bass_kernel_spmd(nc, in_maps, list(range(B)))

    total = 0.0
    for b in range(B):
        rmin = res.results[b]["rowout"].astype(np.float64)   # [128, n_tiles]
        cmin = res.results[b]["colout"].astype(np.float64)   # [128, M]
        sx = np.maximum(rmin, 0.0).sum()
        sy = np.maximum(cmin.min(axis=0), 0.0).sum()
        total += sx / N_PTS + sy / M_PTS
    return np.float32(total / B)



# revision 18
# speedup vs baseline: 1.5772x; 1.0952x over previous
"""Chamfer-distance kernel for 8 Trainium2 NeuronCores (Bass/Tile).

Problem: gts [8, 8192, 3] f32, preds [8, 8192, 3] f32 ->
         scalar chamfer distance (pytorch3d convention: squared L2,
         mean over points, mean over batch, sum of both directions).

Sharding: one batch element per NeuronCore (B == n_cores == 8).

Per-core algorithm (min convention):
  - d2[n, m] = |x_n|^2 + |y_m|^2 - 2 x.y is produced by the PE as a single
    K=24 bf16 matmul per [128 x 512] tile: x/y coordinates are split into
    3-term bf16 hi/mid/lo components (fp32-class accuracy); |x|^2 / |y|^2
    enter as extra contraction rows against a ones-row.  The packed
    operands are replicated at partition offsets 0/32/64/96 so consecutive
    matmuls use different PE row-groups (hides LDWEIGHTS, enables
    concurrent sub-array execution).
  - ScalarE copies each PSUM group [128, 2048] to a bf16 row buffer.
  - VectorE keeps a running elementwise column-min over n-tiles
    (colmin [128, 8192] bf16) and computes row-mins by a halving
    bf16 tensor-tensor min fold over each row block.
  - Host: means + relu clamp + batch mean (epilogue on tiny outputs).
"""

import sys

sys.path.insert(0, "/opt/trn_rl_repo")

import numpy as np
import ml_dtypes

import concourse.bacc as bacc
import concourse.mybir as mybir
import concourse.tile as tile
from concourse.bass_utils import run_bass_kernel_spmd

BF16 = ml_dtypes.bfloat16
POS_BIG = 3.0e38

B = 8
N_PTS = 8192
M_PTS = 8192
K_ROWS = 24
ROT = 4
H_ROWS = 32 * (ROT - 1) + K_ROWS
GROUP = 2048


def _split3(v):
    """float64 vector -> three bf16-representable float64 components."""
    a0 = v.astype(BF16).astype(np.float64)
    a1 = (v - a0).astype(BF16).astype(np.float64)
    a2 = (v - a0 - a1).astype(BF16).astype(np.float64)
    return a0, a1, a2


def _host_pack(x, y):
    """x [N,3] f32, y [M,3] f32 -> xp [H,N] bf16, yp [H,M] bf16 such that
    sum_k xp[k,n]*yp[k,m] = d2(x_n, y_m), replicated at 4 row offsets."""
    N, M = x.shape[0], y.shape[0]
    xd = x.astype(np.float64)
    yd = y.astype(np.float64)
    ax = [_split3(xd[:, d]) for d in range(3)]
    by = [_split3(yd[:, d]) for d in range(3)]
    x2 = _split3((xd * xd).sum(1))
    y2 = _split3((yd * yd).sum(1))

    xp = np.zeros((K_ROWS, N), dtype=BF16)
    yp = np.zeros((K_ROWS, M), dtype=BF16)
    r = 0
    for (i, j) in [(0, 0), (0, 1), (1, 0), (1, 1), (0, 2), (2, 0)]:
        for d in range(3):
            xp[r] = (-2.0 * ax[d][i]).astype(BF16)
            yp[r] = by[d][j].astype(BF16)
            r += 1
    for k in range(3):
        xp[r] = x2[k].astype(BF16)
        yp[r] = np.ones(M, dtype=BF16)
        r += 1
    for k in range(3):
        xp[r] = np.ones(N, dtype=BF16)
        yp[r] = y2[k].astype(BF16)
        r += 1
    assert r == K_ROWS

    xr = np.zeros((H_ROWS, N), dtype=BF16)
    yr = np.zeros((H_ROWS, M), dtype=BF16)
    for q in range(ROT):
        xr[32 * q:32 * q + K_ROWS] = xp
        yr[32 * q:32 * q + K_ROWS] = yp
    return xr, yr


def _build_nc(reps=1):
    f32 = mybir.dt.float32
    bf16 = mybir.dt.bfloat16
    MIN = mybir.AluOpType.min
    nc = bacc.Bacc()
    n_tiles = N_PTS // 128
    n_groups = M_PTS // GROUP
    cpg = GROUP // 512

    xp = nc.declare_dram_parameter("xp", [H_ROWS, N_PTS], bf16, isOutput=False)
    yp = nc.declare_dram_parameter("yp", [H_ROWS, M_PTS], bf16, isOutput=False)
    # rowout holds each tile's rows folded down to width 512; the host does
    # the final 512->1 min (cheaper than the 1x-mode on-device reduce).
    rowout = nc.declare_dram_parameter("rowout", [128, n_tiles * 512], bf16,
                                       isOutput=True)
    colout = nc.declare_dram_parameter("colout", [128, M_PTS], bf16, isOutput=True)

    with tile.TileContext(nc) as tc:
        with (
            tc.tile_pool(name="inputs", bufs=1) as inp,
            tc.tile_pool(name="acc", bufs=1) as acc,
            tc.tile_pool(name="cp", bufs=3) as cpp,
            tc.tile_pool(name="ps", bufs=2, space="PSUM") as psp,
        ):
            xs = inp.tile([H_ROWS, N_PTS], bf16)
            ys = inp.tile([H_ROWS, M_PTS], bf16)
            nc.sync.dma_start(xs[:], xp[:])
            nc.sync.dma_start(ys[:], yp[:])

            colmin = acc.tile([128, M_PTS], bf16)

            def fold_and_ship(cb, tt):
                # halving min-fold (2x bf16 mode) down to width 512, then
                # DMA the partial out on the otherwise-idle Pool queue
                w = M_PTS // 2
                while w >= 512:
                    nc.vector.tensor_tensor(
                        cb[:, :w], cb[:, :w], cb[:, w:2 * w], op=MIN)
                    w //= 2
                nc.gpsimd.dma_start(rowout[:, tt * 512:(tt + 1) * 512],
                                    cb[:, :512])

            def body():
                prev_cbrow = [None]
                for t in range(n_tiles):
                    cbrow = cpp.tile([128, M_PTS], bf16, name="cbrow")
                    for g in range(n_groups):
                        ps = psp.tile([128, GROUP], f32, name="ps")
                        for j in range(cpg):
                            c = g * cpg + j
                            lo = 32 * (c % ROT)
                            nc.tensor.matmul(
                                ps[:, j * 512:(j + 1) * 512],
                                xs[lo:lo + K_ROWS, t * 128:(t + 1) * 128],
                                ys[lo:lo + K_ROWS, c * 512:(c + 1) * 512],
                                start=True,
                                stop=True,
                                tile_position=(lo, 0),
                            )
                        cs = cbrow[:, g * GROUP:(g + 1) * GROUP]
                        nc.scalar.copy(cs, ps[:])
                        cm = colmin[:, g * GROUP:(g + 1) * GROUP]
                        if t == 0:
                            pass  # colmin seeded at t == 1 from both rows
                        elif t == 1:
                            pc = prev_cbrow[0]
                            nc.vector.tensor_tensor(
                                cm, cs, pc[:, g * GROUP:(g + 1) * GROUP],
                                op=MIN)
                        else:
                            nc.vector.tensor_tensor(cm, cs, cm, op=MIN)
                    if t == 0:
                        prev_cbrow[0] = cbrow
                    else:
                        if t == 1:
                            fold_and_ship(prev_cbrow[0], 0)
                            prev_cbrow[0] = None
                        fold_and_ship(cbrow, t)

            if reps == 1:
                body()
            else:
                with tc.For_i(0, reps, 1):
                    body()

            nc.sync.dma_start(colout[:], colmin[:])
    nc.compile()
    return nc


_CACHED_NC = None


def _get_nc():
    global _CACHED_NC
    if _CACHED_NC is None:
        _CACHED_NC = _build_nc(reps=1)
    return _CACHED_NC


def kernel(gts, preds):
    gts = np.asarray(gts, dtype=np.float32)
    preds = np.asarray(preds, dtype=np.float32)
    assert gts.shape == (B, N_PTS, 3) and preds.shape == (B, M_PTS, 3), (
        gts.shape, preds.shape)

    nc = _get_nc()
    in_maps = []
    for b in range(B):
        xp, yp = _host_pack(gts[b], preds[b])
        in_maps.append({"xp": xp, "yp": yp})

    res = run_bass_kernel_spmd(nc, in_maps, list(range(B)))

    n_tiles = N_PTS // 128
    total = 0.0
    for b in range(B):
        rf = res.results[b]["rowout"].astype(np.float64)     # [128, 64*512]
        rmin = rf.reshape(128, n_tiles, 512).min(axis=2)     # [128, n_tiles]
        cmin = res.results[b]["colout"].astype(np.float64)   # [128, M]
        sx = np.maximum(rmin, 0.0).sum()
        sy = np.maximum(cmin.min(axis=0), 0.0).sum()
        total += sx / N_PTS + sy / M_PTS
    return np.float32(total / B)



# revision 19
# speedup vs baseline: 1.6219x; 1.0284x over previous
"""Chamfer-distance kernel for 8 Trainium2 NeuronCores (Bass/Tile).

Problem: gts [8, 8192, 3] f32, preds [8, 8192, 3] f32 ->
         scalar chamfer distance (pytorch3d convention: squared L2,
         mean over points, mean over batch, sum of both directions).

Sharding: one batch element per NeuronCore (B == n_cores == 8).

Per-core algorithm (min convention):
  - d2[n, m] = |x_n|^2 + |y_m|^2 - 2 x.y is produced by the PE as a single
    K=24 bf16 matmul per [128 x 512] tile: x/y coordinates are split into
    3-term bf16 hi/mid/lo components (fp32-class accuracy); |x|^2 / |y|^2
    enter as extra contraction rows against a ones-row.  The packed
    operands are replicated at partition offsets 0/32/64/96 so consecutive
    matmuls use different PE row-groups (hides LDWEIGHTS, enables
    concurrent sub-array execution).
  - ScalarE copies each PSUM group [128, 2048] to a bf16 row buffer.
  - VectorE keeps a running elementwise column-min over n-tiles
    (colmin [128, 8192] bf16) and computes row-mins by a halving
    bf16 tensor-tensor min fold over each row block.
  - Host: means + relu clamp + batch mean (epilogue on tiny outputs).
"""

import sys

sys.path.insert(0, "/opt/trn_rl_repo")

import numpy as np
import ml_dtypes

import concourse.bacc as bacc
import concourse.mybir as mybir
import concourse.tile as tile
from concourse.bass_utils import run_bass_kernel_spmd

BF16 = ml_dtypes.bfloat16
POS_BIG = 3.0e38

B = 8
N_PTS = 8192
M_PTS = 8192
K_ROWS = 24
ROT = 4
H_ROWS = 32 * (ROT - 1) + K_ROWS
GROUP = 2048


def _split3(v):
    """float64 vector -> three bf16-representable float64 components."""
    a0 = v.astype(BF16).astype(np.float64)
    a1 = (v - a0).astype(BF16).astype(np.float64)
    a2 = (v - a0 - a1).astype(BF16).astype(np.float64)
    return a0, a1, a2


def _host_pack(x, y):
    """x [N,3] f32, y [M,3] f32 -> xp [H,N] bf16, yp [H,M] bf16 such that
    sum_k xp[k,n]*yp[k,m] = d2(x_n, y_m), replicated at 4 row offsets."""
    N, M = x.shape[0], y.shape[0]
    xd = x.astype(np.float64)
    yd = y.astype(np.float64)
    ax = [_split3(xd[:, d]) for d in range(3)]
    by = [_split3(yd[:, d]) for d in range(3)]
    x2 = _split3((xd * xd).sum(1))
    y2 = _split3((yd * yd).sum(1))

    xp = np.zeros((K_ROWS, N), dtype=BF16)
    yp = np.zeros((K_ROWS, M), dtype=BF16)
    r = 0
    for (i, j) in [(0, 0), (0, 1), (1, 0), (1, 1), (0, 2), (2, 0)]:
        for d in range(3):
            xp[r] = (-2.0 * ax[d][i]).astype(BF16)
            yp[r] = by[d][j].astype(BF16)
            r += 1
    for k in range(3):
        xp[r] = x2[k].astype(BF16)
        yp[r] = np.ones(M, dtype=BF16)
        r += 1
    for k in range(3):
        xp[r] = np.ones(N, dtype=BF16)
        yp[r] = y2[k].astype(BF16)
        r += 1
    assert r == K_ROWS

    xr = np.zeros((H_ROWS, N), dtype=BF16)
    yr = np.zeros((H_ROWS, M), dtype=BF16)
    for q in range(ROT):
        xr[32 * q:32 * q + K_ROWS] = xp
        yr[32 * q:32 * q + K_ROWS] = yp
    return xr, yr


def _build_nc(reps=1):
    f32 = mybir.dt.float32
    bf16 = mybir.dt.bfloat16
    MIN = mybir.AluOpType.min
    nc = bacc.Bacc()
    n_tiles = N_PTS // 128
    n_groups = M_PTS // GROUP
    cpg = GROUP // 512

    xp = nc.declare_dram_parameter("xp", [H_ROWS, N_PTS], bf16, isOutput=False)
    yp = nc.declare_dram_parameter("yp", [H_ROWS, M_PTS], bf16, isOutput=False)
    # rowout holds each tile's rows folded down to width 512; the host does
    # the final 512->1 min (cheaper than the 1x-mode on-device reduce).
    rowout = nc.declare_dram_parameter("rowout", [128, n_tiles * 512], bf16,
                                       isOutput=True)
    colout = nc.declare_dram_parameter("colout", [128, M_PTS], bf16, isOutput=True)

    with tile.TileContext(nc) as tc:
        with (
            tc.tile_pool(name="inputs", bufs=1) as inp,
            tc.tile_pool(name="acc", bufs=1) as acc,
            tc.tile_pool(name="cp", bufs=3) as cpp,
            tc.tile_pool(name="ps", bufs=2, space="PSUM") as psp,
        ):
            xs = inp.tile([H_ROWS, N_PTS], bf16)
            ys = inp.tile([H_ROWS, M_PTS], bf16)
            nc.sync.dma_start(xs[:], xp[:])
            nc.sync.dma_start(ys[:], yp[:])

            colmin = acc.tile([128, M_PTS], bf16)

            def body():
                wide_ref = [None]
                for t in range(n_tiles):
                    h = t % 2
                    if h == 0:
                        wide_ref[0] = cpp.tile([128, 2 * M_PTS], bf16,
                                               name="wide")
                    wide = wide_ref[0]
                    base = h * M_PTS
                    for g in range(n_groups):
                        ps = psp.tile([128, GROUP], f32, name="ps")
                        for j in range(cpg):
                            c = g * cpg + j
                            lo = 32 * (c % ROT)
                            nc.tensor.matmul(
                                ps[:, j * 512:(j + 1) * 512],
                                xs[lo:lo + K_ROWS, t * 128:(t + 1) * 128],
                                ys[lo:lo + K_ROWS, c * 512:(c + 1) * 512],
                                start=True,
                                stop=True,
                                tile_position=(lo, 0),
                            )
                        cs = wide[:, base + g * GROUP:base + (g + 1) * GROUP]
                        nc.scalar.copy(cs, ps[:])
                        cm = colmin[:, g * GROUP:(g + 1) * GROUP]
                        if t == 0:
                            pass  # colmin seeded at t == 1 from both rows
                        elif t == 1:
                            nc.vector.tensor_tensor(
                                cm, cs, wide[:, g * GROUP:(g + 1) * GROUP],
                                op=MIN)
                        else:
                            nc.vector.tensor_tensor(cm, cs, cm, op=MIN)
                    if h == 1:
                        # pair-batched halving min-fold on both rows at once
                        view = wide[:].rearrange("p (a b) -> p a b", a=2)
                        w = M_PTS // 2
                        while w >= 512:
                            nc.vector.tensor_tensor(
                                view[:, :, :w], view[:, :, :w],
                                view[:, :, w:2 * w], op=MIN)
                            w //= 2
                        nc.gpsimd.dma_start(
                            rowout[:, (t - 1) * 512:(t + 1) * 512],
                            view[:, :, :512])

            if reps == 1:
                body()
            else:
                with tc.For_i(0, reps, 1):
                    body()

            nc.sync.dma_start(colout[:], colmin[:])
    nc.compile()
    return nc


_CACHED_NC = None


def _get_nc():
    global _CACHED_NC
    if _CACHED_NC is None:
        _CACHED_NC = _build_nc(reps=1)
    return _CACHED_NC


def kernel(gts, preds):
    gts = np.asarray(gts, dtype=np.float32)
    preds = np.asarray(preds, dtype=np.float32)
    assert gts.shape == (B, N_PTS, 3) and preds.shape == (B, M_PTS, 3), (
        gts.shape, preds.shape)

    nc = _get_nc()
    in_maps = []
    for b in range(B):
        xp, yp = _host_pack(gts[b], preds[b])
        in_maps.append({"xp": xp, "yp": yp})

    res = run_bass_kernel_spmd(nc, in_maps, list(range(B)))

    n_tiles = N_PTS // 128
    total = 0.0
    for b in range(B):
        rf = res.results[b]["rowout"].astype(np.float64)     # [128, 64*512]
        rmin = rf.reshape(128, n_tiles, 512).min(axis=2)     # [128, n_tiles]
        cmin = res.results[b]["colout"].astype(np.float64)   # [128, M]
        sx = np.maximum(rmin, 0.0).sum()
        sy = np.maximum(cmin.min(axis=0), 0.0).sum()
        total += sx / N_PTS + sy / M_PTS
    return np.float32(total / B)

